# revision 17
# baseline (speedup 1.0000x reference)
"""Trainium2 Bass kernel for bidirectional Mamba2 (DSSMamba2), 8 NeuronCores.

Strategy (single SPMD launch, all-bf16 matmul datapath):
  Phase 1 (all 8 cores): each core owns 4 fwd + 4 bwd heads. Computes its
  in_proj slice (channels-on-partitions, bf16 weights loaded once per
  m-tile in a host-prearranged contiguous layout), depthwise causal /
  anti-causal conv + silu, softplus(dt), and the chunked-SSD scan
  (8 chunks of 128) for its heads. Backward direction is evaluated in
  reverse ("true") time so no flips are ever materialized. ln(dt_s) is
  folded into the exp() bias so T = G * exp(cum_t - cum_s + ln dt_s + C0)
  needs one ACT exp + one DVE mult per head-chunk. Gated outputs
  yg = y * silu(z) are written per chunk as bf16 and redistributed
  head-split -> L-split with two bf16 AllToAlls.
  Phase 2 runs per piece (gated RMSNorm, out_proj, silu, final
  projection); piece 0 starts as soon as its AllToAll lands, hiding the
  second AllToAll. out_proj / final weights are preloaded into SBUF at
  kernel start so phase 2 is pure compute.
  Host: reorders/adds the per-core partials (pure gather/unshard).
"""
import sys
if '/opt/trn_rl_repo' not in sys.path:
    sys.path.insert(0, '/opt/trn_rl_repo')

import numpy as np
import ml_dtypes
import concourse.bass as bass
import concourse.tile as tile
from concourse import bacc, mybir
from concourse.bass_utils import run_bass_kernel_spmd

F32 = mybir.dt.float32
BF16 = mybir.dt.bfloat16
AOP = mybir.AluOpType
ACT = mybir.ActivationFunctionType
NPBF = np.dtype(ml_dtypes.bfloat16)

L = 1024
DM = 1024            # d_model
Q = 128              # chunk length
NCH = 8              # number of chunks
P = 64               # headdim
NCORES = 8
CIN = 1288           # per-core in_proj rows: 2*(256 z + 256 x + 128 BC + 4 dt)
EPS = 1e-5
D_INNER = 2048

_CACHE = {}


def _build():
    nc = bacc.Bacc("TRN2", target_bir_lowering=False, debug=False)

    # ---- DRAM I/O ----
    uT = nc.dram_tensor("uT", [DM, L], BF16, kind="ExternalInput")
    # winT cols: m-blocks in order [dt(8), m0..m9(128 each)], each block
    # k-major: col = off(m) + k*mrows + c
    winT = nc.dram_tensor("winT", [128, 8 * CIN], BF16, kind="ExternalInput")
    convw = nc.dram_tensor("convw", [128, 24], F32, kind="ExternalInput")
    convb = nc.dram_tensor("convb", [128, 6], F32, kind="ExternalInput")
    dtbias = nc.dram_tensor("dtbias", [8, 1], F32, kind="ExternalInput")
    arow = nc.dram_tensor("arow", [128, 8], F32, kind="ExternalInput")
    dcol = nc.dram_tensor("dcol", [128, 4], F32, kind="ExternalInput")
    tri = nc.dram_tensor("tri", [128, 512], BF16, kind="ExternalInput")
    # tri columns: [TRI_f | TRI_b | TRIR_f | TRIR_b]
    c0m = nc.dram_tensor("c0m", [128, 1024], BF16, kind="ExternalInput")
    # c0m: [C0_f x4 | C0_b x4]  (per-head-tiled mask)
    ident = nc.dram_tensor("ident", [128, 128], F32, kind="ExternalInput")
    identb = nc.dram_tensor("identb", [128, 128], BF16, kind="ExternalInput")
    onescol = nc.dram_tensor("onescol", [128, 1], BF16, kind="ExternalInput")
    onesrow = nc.dram_tensor("onesrow", [1, 128], BF16, kind="ExternalInput")
    # woT cols: m(8) x k(16) x c(128); wfT cols: m2(8) x k(8) x c(128)
    woT = nc.dram_tensor("woT", [128, 16384], BF16, kind="ExternalInput")
    wfT = nc.dram_tensor("wfT", [128, 8192], BF16, kind="ExternalInput")
    out_part = nc.dram_tensor("out_part", [DM, 256], F32, kind="ExternalOutput")

    # collective bounce buffers (internal DRAM)
    a2a_in = [nc.dram_tensor(f"a2a_in{p}", [NCORES, 256, Q], BF16)
              for p in range(2)]
    a2a_out = [nc.dram_tensor(f"a2a_out{p}", [NCORES, 256, Q], BF16)
               for p in range(2)]

    with tile.TileContext(nc) as tc, \
            nc.allow_low_precision(reason="bf16 datapath within 2e-2 budget"):
        import contextlib
        ctx = contextlib.ExitStack()
        sb = ctx.enter_context(tc.tile_pool(name="sb", bufs=1))
        scr = ctx.enter_context(tc.tile_pool(name="scr", bufs=3))
        scs = ctx.enter_context(tc.tile_pool(name="scs", bufs=5))
        zpool = ctx.enter_context(tc.tile_pool(name="zpool", bufs=4))
        upool = ctx.enter_context(tc.tile_pool(name="upool", bufs=4))
        blpool = ctx.enter_context(tc.tile_pool(name="blpool", bufs=2))
        bcp = ctx.enter_context(tc.tile_pool(name="bcp", bufs=4))
        w_pool = ctx.enter_context(tc.tile_pool(name="wstream", bufs=2))
        uT_pool = ctx.enter_context(tc.tile_pool(name="uTp", bufs=8))
        yga_pool = ctx.enter_context(tc.tile_pool(name="yga", bufs=24))
        ps_mm = ctx.enter_context(tc.tile_pool(name="ps_mm", bufs=2, space="PSUM"))
        ps_gt = ctx.enter_context(tc.tile_pool(name="ps_gt", bufs=2, space="PSUM"))
        ps_y = ctx.enter_context(tc.tile_pool(name="ps_y", bufs=2, space="PSUM"))
        ps_cc = ctx.enter_context(tc.tile_pool(name="ps_cc", bufs=1, space="PSUM"))

        # ---- uT into SBUF (scalar ring, ahead of the weight preload) ----
        uT_t = []
        for k in range(8):
            t = uT_pool.tile([128, 1024], BF16, tag="uT", name=f"uTk{k}")
            nc.scalar.dma_start(out=t[:], in_=uT[k * 128:(k + 1) * 128, :])
            uT_t.append(t)

        # ---- phase-2 weights preloaded (stream during phase 1) ----
        wo_sb = sb.tile([128, 16384], BF16, tag="wo")
        wf_sb = sb.tile([128, 8192], BF16, tag="wf")
        for q in range(4):
            nc.scalar.dma_start(out=wo_sb[:, q * 4096:(q + 1) * 4096],
                                in_=woT[:, q * 4096:(q + 1) * 4096])
            nc.scalar.dma_start(out=wf_sb[:, q * 2048:(q + 1) * 2048],
                                in_=wfT[:, q * 2048:(q + 1) * 2048])


        # ---- constants into SBUF ----
        tri_t = sb.tile([128, 512], BF16, tag="tri")
        nc.sync.dma_start(out=tri_t[:], in_=tri[:])
        c0_t = sb.tile([128, 1024], BF16, tag="c0")
        nc.sync.dma_start(out=c0_t[:], in_=c0m[:])
        id_t = sb.tile([128, 128], F32, tag="id")
        nc.sync.dma_start(out=id_t[:], in_=ident[:])
        idb_t = sb.tile([128, 128], BF16, tag="idb")
        nc.sync.dma_start(out=idb_t[:], in_=identb[:])
        ones_t = sb.tile([128, 1], BF16, tag="ones")
        nc.sync.dma_start(out=ones_t[:], in_=onescol[:])
        onesr_t = sb.tile([1, 128], BF16, tag="onesr")
        nc.sync.dma_start(out=onesr_t[:], in_=onesrow[:])
        cw_t = sb.tile([128, 24], F32, tag="cw")
        nc.sync.dma_start(out=cw_t[:], in_=convw[:])
        cb_t = sb.tile([128, 6], F32, tag="cb")
        nc.sync.dma_start(out=cb_t[:], in_=convb[:])
        dtb_t = sb.tile([8, 1], F32, tag="dtb")
        nc.sync.dma_start(out=dtb_t[:], in_=dtbias[:])
        ar_t = sb.tile([128, 8], F32, tag="ar")
        nc.sync.dma_start(out=ar_t[:], in_=arow[:])
        dc_t = sb.tile([128, 4], F32, tag="dc")
        nc.sync.dma_start(out=dc_t[:], in_=dcol[:])
        one8 = sb.tile([8, 1], F32, tag="one8")
        nc.vector.memset(one8[:], 1.0)
        onesrf = sb.tile([1, 128], F32, tag="onesrf")
        nc.vector.memset(onesrf[:], 1.0)

        TRI_f = tri_t[:, 0:128]
        TRI_b = tri_t[:, 128:256]
        TRIR_f = tri_t[:, 256:384]
        TRIR_b = tri_t[:, 384:512]
        C04 = [c0_t[:, 0:512], c0_t[:, 512:1024]]

        # ---- persistent phase-1 tiles ----
        z_sb = [zpool.tile([128, L], BF16, tag="z", name=f"z{t}")
                for t in range(4)]
        xpad = [sb.tile([128, L + 3], BF16, tag=f"xpad{t}", name=f"xpad{t}")
                for t in range(6)]
        xact = [sb.tile([128, L], BF16, tag=f"xact{t}", name=f"xact{t}")
                for t in range(6)]
        cact = [sb.tile([64, L], BF16, tag=f"cact{d}", name=f"cact{d}")
                for d in range(2)]
        ygs = [sb.tile([128, L], BF16, tag=f"ygs{t}", name=f"ygs{t}")
               for t in range(4)]
        dt_sb = sb.tile([8, L], F32, tag="dt")
        cum8 = sb.tile([8, L], F32, tag="cum8")
        wrev8 = sb.tile([8, L], F32, tag="wrev8")
        lndt8 = sb.tile([8, L], F32, tag="lndt8")
        sd8 = sb.tile([8, L], F32, tag="sd8")
        omdt8 = sb.tile([8, L], BF16, tag="omdt8")
        ecum8 = sb.tile([8, L], BF16, tag="ecum8")
        ecum8f = sb.tile([8, L], F32, tag="ecum8f")
        edrow = sb.tile([1, 64], F32, tag="edrow")
        edall = sb.tile([64, 64], F32, tag="edall")
        sdT_sb = [sb.tile([128, 8], F32, tag=f"sdT{c}", name=f"sdT{c}")
                  for c in range(NCH)]
        omdtT_sb = [sb.tile([128, 8], F32, tag=f"om{c}", name=f"om{c}")
                    for c in range(NCH)]
        S_sb = [sb.tile([64, 64], BF16, tag=f"S{h}", name=f"S{h}")
                for h in range(8)]

        # zero-pad columns of conv inputs
        for t in range(6):
            fwd = t in (0, 1, 4)
            if fwd:
                nc.vector.memset(xpad[t][:, 0:3], 0.0)
            else:
                nc.vector.memset(xpad[t][:, L:L + 3], 0.0)
        for h in range(8):
            nc.vector.memset(S_sb[h][:], 0.0)

        # ---- in_proj: out^T[m-tile, L] = winT.T @ uT ----
        # m-tile map: 0-1 zf, 2-3 zb, 4-5 xf, 6-7 xb, 8 BCf, 9 BCb, 10 dt(8)
        off = {}
        o = 0
        for m in [10] + list(range(10)):
            off[m] = o
            o += 8 * (8 if m == 10 else 128)
        for m in [10] + list(range(10)):
            mrows = 8 if m == 10 else 128
            wt = w_pool.tile([128, 8, mrows], BF16, tag="w", name="wct")
            nc.sync.dma_start(out=wt[:],
                              in_=winT[:, off[m]:off[m] + 8 * mrows])
            ps = [ps_mm.tile([128, 512], F32, tag="mm", name=f"mmps{nh}")
                  for nh in range(2)]
            for k in range(8):
                for nh in range(2):
                    nc.tensor.matmul(
                        ps[nh][0:mrows, :], wt[:, k, :],
                        uT_t[k][:, nh * 512:(nh + 1) * 512],
                        start=(k == 0), stop=(k == 7))
            for nh in range(2):
                nsl = slice(nh * 512, (nh + 1) * 512)
                if m == 10:     # dt: softplus(dt + bias) = ln(1 + exp(.))
                    spe = scr.tile([8, 512], F32, tag="scr8", name="spe", bufs=2)
                    nc.scalar.activation(
                        out=spe[:], in_=ps[nh][0:8, :],
                        func=ACT.Exp, bias=dtb_t[:], scale=1.0)
                    nc.scalar.activation(
                        out=dt_sb[:, nsl], in_=spe[:],
                        func=ACT.Ln, bias=one8[:], scale=1.0)
                elif m < 4:     # z
                    nc.scalar.activation(out=z_sb[m][:, nsl], in_=ps[nh][:],
                                         func=ACT.Copy)
                else:           # xBC -> padded tile
                    t = m - 4
                    po = 3 if t in (0, 1, 4) else 0
                    nc.scalar.activation(
                        out=xpad[t][:, po + nh * 512: po + (nh + 1) * 512],
                        in_=ps[nh][:], func=ACT.Copy)

        # ---- silu(z) in place (z is only ever used gated) ----
        for t in range(4):
            for hp2 in range(2):
                zsl = slice(hp2 * 512, (hp2 + 1) * 512)
                sgz = scr.tile([128, 512], BF16, tag="scr", name="sgz")
                nc.scalar.activation(out=sgz[:], in_=z_sb[t][:, zsl],
                                     func=ACT.Sigmoid)
                nc.vector.tensor_tensor(out=z_sb[t][:, zsl],
                                        in0=z_sb[t][:, zsl], in1=sgz[:],
                                        op=AOP.mult)

        # ---- conv + silu ----
        for t in range(6):
            fwd = t in (0, 1, 4)
            eng = nc.vector
            cvo = scr.tile([128, L], BF16, tag="scr", name="cvo")
            first = True
            for w in range(4):
                po = w if fwd else 3 - w
                src = xpad[t][:, po:po + L]
                wcol = cw_t[:, t * 4 + w:t * 4 + w + 1]
                if first:
                    eng.tensor_scalar(
                        out=cvo[:], in0=src, scalar1=wcol,
                        scalar2=cb_t[:, t:t + 1], op0=AOP.mult, op1=AOP.add)
                    first = False
                else:
                    eng.scalar_tensor_tensor(
                        out=cvo[:], in0=src, scalar=wcol, in1=cvo[:],
                        op0=AOP.mult, op1=AOP.add)
            nc.scalar.activation(out=xact[t][:], in_=cvo[:], func=ACT.Sigmoid)
            nc.vector.tensor_tensor(out=xact[t][:], in0=xact[t][:],
                                    in1=cvo[:], op=AOP.mult)
            if t >= 4:
                nc.sync.dma_start(out=cact[t - 4][:], in_=xact[t][64:128, :])

        # ---- dt preprocessing ----
        nc.scalar.activation(out=lndt8[:], in_=dt_sb[:], func=ACT.Ln)
        for c in range(NCH):
            csl = slice(c * Q, (c + 1) * Q)
            tp = ps_gt.tile([128, 128], F32, tag="gt", name="tp")
            nc.tensor.transpose(tp[:, 0:8], dt_sb[:, csl], id_t[0:8, 0:8])
            ldT = scs.tile([128, 8], BF16, tag="ldT", name="ldT")
            nc.vector.tensor_tensor(out=ldT[:], in0=tp[:, 0:8],
                                    in1=ar_t[:], op=AOP.mult)
            ccps = ps_cc.tile([4, 512], F32, tag="cc", name="ccps")
            nc.tensor.matmul(ccps[0:4, 0:128], ldT[:, 0:4], TRI_f,
                             start=True, stop=True)
            nc.tensor.matmul(ccps[0:4, 128:256], ldT[:, 0:4], TRIR_f,
                             start=True, stop=True)
            nc.tensor.matmul(ccps[0:4, 256:384], ldT[:, 4:8], TRI_b,
                             start=True, stop=True)
            nc.tensor.matmul(ccps[0:4, 384:512], ldT[:, 4:8], TRIR_b,
                             start=True, stop=True)
            stg = scr.tile([4, 512], F32, tag="stg", name="stg", bufs=2)
            nc.vector.tensor_copy(out=stg[:], in_=ccps[0:4, :])
            # rows 4:8 are partition-unaligned for engines; DMA is exempt
            nc.sync.dma_start(out=cum8[0:4, csl], in_=stg[0:4, 0:128])
            nc.sync.dma_start(out=cum8[4:8, csl], in_=stg[0:4, 256:384])
            nc.sync.dma_start(out=wrev8[0:4, csl], in_=stg[0:4, 128:256])
            nc.sync.dma_start(out=wrev8[4:8, csl], in_=stg[0:4, 384:512])
        # sd = ln(dt) - cum ; omdt = exp(wrev + ln dt) ; ecum = exp(cum)
        nc.vector.tensor_tensor(out=sd8[:], in0=lndt8[:], in1=cum8[:],
                                op=AOP.subtract)
        nc.vector.tensor_tensor(out=wrev8[:], in0=wrev8[:], in1=lndt8[:],
                                op=AOP.add)
        nc.scalar.activation(out=omdt8[:], in_=wrev8[:], func=ACT.Exp)
        nc.scalar.activation(out=ecum8[:], in_=cum8[:], func=ACT.Exp)
        nc.scalar.activation(out=ecum8f[:], in_=cum8[:], func=ACT.Exp)
        # per-chunk decay scalars exp(cum at boundary); fwd boundary is
        # chunk position 127, bwd position 0. Gather all 64 (c,h) values
        # into one row, broadcast once.
        for c in range(NCH):
            nc.sync.dma_start(
                out=edrow[0:1, c * 8:c * 8 + 4],
                in_=ecum8f[0:4, c * Q + Q - 1:c * Q + Q])
            nc.sync.dma_start(
                out=edrow[0:1, c * 8 + 4:c * 8 + 8],
                in_=ecum8f[4:8, c * Q:c * Q + 1])
        nc.gpsimd.partition_broadcast(edall[:], edrow[0:1, :])
        for c in range(NCH):
            csl = slice(c * Q, (c + 1) * Q)
            tps = ps_gt.tile([128, 128], F32, tag="gt", name="tps")
            nc.tensor.transpose(tps[:, 0:8], sd8[:, csl], id_t[0:8, 0:8])
            nc.vector.tensor_copy(out=sdT_sb[c][:], in_=tps[:, 0:8])
            tpo = ps_gt.tile([128, 128], BF16, tag="gt", name="tpo")
            nc.tensor.transpose(tpo[:, 0:8], omdt8[:, csl], idb_t[0:8, 0:8])
            nc.vector.tensor_copy(out=omdtT_sb[c][:], in_=tpo[:, 0:8])

        # ---- chunked SSD scan ----
        def make_bcast(d, c):
            """Per-head cum + exp(cum) rows for chunk c broadcast to 128
            partitions. bc4 additionally gets the C0 causal mask added."""
            csl = slice(c * Q, (c + 1) * Q)
            bc = bcp.tile([128, 512], F32, tag="bc", name="bc")
            eb = bcp.tile([128, 512], BF16, tag="eb", name="eb")
            bcr = bcp.tile([1, 512], F32, tag="bcr", name="bcr", bufs=2)
            ebr = bcp.tile([1, 512], BF16, tag="ebr", name="ebr", bufs=2)
            nc.sync.dma_start(out=bcr[0:1, :],
                              in_=cum8[d * 4:d * 4 + 4, csl])
            nc.sync.dma_start(out=ebr[0:1, :],
                              in_=ecum8[d * 4:d * 4 + 4, csl])
            # broadcast rows to 128 partitions via rank-1 matmuls
            bps = ps_mm.tile([128, 512], F32, tag="mm", name="bps")
            nc.tensor.matmul(bps[:], onesrf[:], bcr[:], start=True, stop=True)
            nc.vector.tensor_tensor(out=bc[:], in0=bps[:], in1=C04[d],
                                    op=AOP.add)
            eps_ = ps_mm.tile([128, 512], F32, tag="mm", name="ebps")
            nc.tensor.matmul(eps_[:], onesr_t[:], ebr[:],
                             start=True, stop=True)
            nc.scalar.activation(out=eb[:], in_=eps_[:], func=ACT.Copy)
            return bc, eb

        def scan_chunk(d, c, first, bc, eb):
            """Process direction d (0 fwd, 1 bwd) chunk c (true time)."""
            csl = slice(c * Q, (c + 1) * Q)
            BC = xact[4 + d]
            # B transpose (L-on-partitions) for state accumulation
            btp = ps_gt.tile([128, 128], BF16, tag="gt", name="btp")
            nc.tensor.transpose(btp[:, 0:64], BC[0:64, csl],
                                idb_t[0:64, 0:64])
            bl = blpool.tile([128, 64], BF16, tag="bl", name="bl")
            nc.scalar.activation(out=bl[:], in_=btp[:, 0:64], func=ACT.Copy)
            # x^T transposes
            u_t = []
            for xi in range(2):
                xt = 2 * d + xi
                up = ps_gt.tile([128, 128], BF16, tag="gt", name="up")
                nc.tensor.transpose(up[:], xact[xt][:, csl], idb_t[:])
                ut = upool.tile([128, 128], BF16, tag="u", name="ut")
                nc.scalar.activation(out=ut[:], in_=up[:], func=ACT.Copy)
                u_t.append(ut)
            # G (shared across the dir's heads)
            gps = ps_gt.tile([128, 128], F32, tag="gt", name="gps")
            nc.tensor.matmul(gps[:], BC[0:64, csl], cact[d][:, csl],
                             start=True, stop=True)
            gsb = scs.tile([128, 128], BF16, tag="gsb", name="gsb")
            nc.scalar.activation(out=gsb[:], in_=gps[:], func=ACT.Copy)
            # batched states for all 4 heads: uwall = x^T * (dt*omega)
            uwall = scr.tile([128, 256], BF16, tag="scrh", name="uwall")
            for hl in range(4):
                col = d * 4 + hl
                nc.vector.tensor_scalar_mul(
                    out=uwall[:, hl * 64:(hl + 1) * 64],
                    in0=u_t[hl // 2][:, (hl % 2) * 64:(hl % 2) * 64 + 64],
                    scalar1=omdtT_sb[c][:, col:col + 1])
            sps4 = ps_y.tile([64, 256], F32, tag="st", name="sps4", bufs=1)
            nc.tensor.matmul(sps4[:], bl[:], uwall[:], start=True, stop=True)
            for xi in range(2):
                xt = 2 * d + xi
                yful = ps_y.tile([128, 128], F32, tag="y", name="yful")
                for hh in range(2):
                    col = d * 4 + xi * 2 + hh
                    hl4 = xi * 2 + hh
                    hsl = slice(hh * 64, (hh + 1) * 64)
                    # T = G * exp(cum_t - cum_s + ln dt_s + C0)
                    ee = scs.tile([128, 128], BF16, tag="ee", name="ee")
                    nc.scalar.activation(
                        out=ee[:], in_=bc[:, hl4 * Q:(hl4 + 1) * Q],
                        func=ACT.Exp, bias=sdT_sb[c][:, col:col + 1],
                        scale=1.0)
                    tt = scs.tile([128, 128], BF16, tag="tt", name="ttl")
                    nc.vector.tensor_tensor(out=tt[:], in0=ee[:],
                                            in1=gsb[:], op=AOP.mult)
                    # Y^T = U^T T (+ S^T (C*ecum))
                    nc.tensor.matmul(yful[hsl, :], u_t[xi][:, hsl], tt[:],
                                     start=True, stop=first)
                    if not first:
                        cem = scs.tile([64, 128], BF16, tag="cem",
                                       name="cem")
                        nc.vector.tensor_tensor(
                            out=cem[:], in0=cact[d][:, csl],
                            in1=eb[0:64, hl4 * Q:(hl4 + 1) * Q],
                            op=AOP.mult)
                        nc.tensor.matmul(yful[hsl, :], S_sb[col][:], cem[:],
                                         start=False, stop=True)
                    # S = edecay * S + states  (after Y read S)
                    nc.vector.scalar_tensor_tensor(
                        out=S_sb[col][:], in0=S_sb[col][:],
                        scalar=edall[:, c * 8 + col:c * 8 + col + 1],
                        in1=sps4[:, hl4 * 64:(hl4 + 1) * 64],
                        op0=AOP.mult, op1=AOP.add)
                # y = Y + D*x, then gate with silu(z), bf16 out
                ysc = scs.tile([128, 128], BF16, tag="ysc", name="ysc")
                nc.vector.scalar_tensor_tensor(
                    out=ysc[:], in0=xact[xt][:, csl],
                    scalar=dc_t[:, xt:xt + 1], in1=yful[:],
                    op0=AOP.mult, op1=AOP.add)
                nc.vector.tensor_tensor(out=ygs[xt][:, csl], in0=ysc[:],
                                        in1=z_sb[xt][:, csl], op=AOP.mult)

        def emit_piece(p):
            """Stage already-gated yg columns, launch AllToAll."""
            # piece 0: fwd chunks 0-3 to cores 0-3, bwd chunks 4-7 to 4-7
            for k in range(NCORES):
                if k < 4:
                    ch = k if p == 0 else k + 4     # fwd chunk
                    t0, t1 = 0, 1
                else:
                    j = k - 4
                    ch = (j + 4) if p == 0 else j   # bwd chunk
                    t0, t1 = 2, 3
                nc.scalar.dma_start(out=a2a_in[p][k, 0:128, :],
                                    in_=ygs[t0][:, ch * Q:(ch + 1) * Q])
                nc.scalar.dma_start(out=a2a_in[p][k, 128:256, :],
                                    in_=ygs[t1][:, ch * Q:(ch + 1) * Q])
            nc.gpsimd.collective_compute(
                "AllToAll", AOP.bypass,
                replica_groups=[list(range(NCORES))],
                ins=[a2a_in[p].ap().opt()], outs=[a2a_out[p].ap().opt()])

        bc_next = [make_bcast(0, 0), make_bcast(1, NCH - 1)]
        for step in range(NCH):
            bc_cur = bc_next
            if step + 1 < NCH:
                bc_next = [make_bcast(0, step + 1),
                           make_bcast(1, NCH - 2 - step)]
            scan_chunk(0, step, first=(step == 0),
                       bc=bc_cur[0][0], eb=bc_cur[0][1])
            scan_chunk(1, NCH - 1 - step, first=(step == 0),
                       bc=bc_cur[1][0], eb=bc_cur[1][1])
            if step == 3:
                emit_piece(0)
        emit_piece(1)

        # ---- phase 2 (per piece): receive, norm, out_proj, silu, final ----
        epst = sb.tile([1, 1], F32, tag="epst")
        nc.vector.memset(epst[:], EPS)
        for p in range(2):
            ygall = []
            for g in range(16):
                t = yga_pool.tile([128, 128], BF16, tag="yga",
                                  name=f"yga{p}_{g}")
                nc.scalar.dma_start(
                    out=t[:], in_=a2a_out[p][g // 2,
                                             (g % 2) * 128:(g % 2) * 128 + 128,
                                             :])
                ygall.append(t)
            ssps = ps_cc.tile([1, 512], F32, tag="cc", name="ssps")
            for g in range(16):
                sq = scs.tile([128, 128], BF16, tag="sq", name="sq")
                nc.vector.tensor_tensor(out=sq[:], in0=ygall[g][:],
                                        in1=ygall[g][:], op=AOP.mult)
                nc.tensor.matmul(ssps[0:1, 0:128], ones_t[:], sq[:],
                                 start=(g == 0), stop=(g == 15))
            rs = scs.tile([1, 128], BF16, tag="rs", name="rs")
            rsf = scs.tile([1, 128], F32, tag="rsf", name="rsf")
            nc.scalar.activation(out=rsf[:], in_=ssps[0:1, 0:128],
                                 func=ACT.Sqrt, bias=epst[:],
                                 scale=1.0 / D_INNER)
            nc.vector.reciprocal(out=rs[:], in_=rsf[:])
            rsb = scs.tile([128, 128], BF16, tag="rsb", name="rsb")
            nc.gpsimd.partition_broadcast(rsb[:], rs[0:1, :])

            silu_sb = []
            for m in range(8):
                ps = ps_mm.tile([128, 128], F32, tag="mm", name="mm2")
                for kk in range(16):
                    nc.tensor.matmul(
                        ps[:], wo_sb[:, m * 2048 + kk * 128:
                                     m * 2048 + (kk + 1) * 128],
                        ygall[kk][:], start=(kk == 0), stop=(kk == 15))
                t1 = scs.tile([128, 128], BF16, tag="t1", name="t1")
                nc.vector.tensor_tensor(out=t1[:], in0=ps[:], in1=rsb[:],
                                        op=AOP.mult)
                sg = scs.tile([128, 128], BF16, tag="sg", name="sg")
                nc.scalar.activation(out=sg[:], in_=t1[:], func=ACT.Sigmoid)
                st = yga_pool.tile([128, 128], BF16, tag="yga",
                                   name=f"st{p}_{m}")
                nc.vector.tensor_tensor(out=st[:], in0=sg[:],
                                        in1=t1[:], op=AOP.mult)
                silu_sb.append(st)

            for m2 in range(8):
                ps = ps_mm.tile([128, 128], F32, tag="mm", name="mm3")
                for m in range(8):
                    nc.tensor.matmul(
                        ps[:], wf_sb[:, m2 * 1024 + m * 128:
                                     m2 * 1024 + (m + 1) * 128],
                        silu_sb[m][:], start=(m == 0), stop=(m == 7))
                ot = scs.tile([128, 128], F32, tag="ot", name="ot")
                nc.scalar.activation(out=ot[:], in_=ps[:], func=ACT.Copy)
                nc.sync.dma_start(
                    out=out_part[m2 * 128:(m2 + 1) * 128,
                                 p * 128:(p + 1) * 128],
                    in_=ot[:])
        ctx.close()

    nc.compile()
    return nc


def _prep_inputs(u, W_in_f, W_in_b, conv_w_f, conv_b_f, conv_w_b, conv_b_b,
                 dt_bias_f, dt_bias_b, A_log_f, A_log_b, D_f, D_b,
                 norm_w_f, norm_w_b, W_out_f, W_out_b, W_out):
    f32 = np.float32
    uT = np.ascontiguousarray(u[0].T).astype(NPBF)
    r = np.arange(Q)
    TRI_f = (r[:, None] <= r[None, :]).astype(f32)
    TRI_b = (r[:, None] >= r[None, :]).astype(f32)
    TRIR_f = (r[:, None] > r[None, :]).astype(f32)
    TRIR_b = (r[:, None] < r[None, :]).astype(f32)
    tri = np.concatenate([TRI_f, TRI_b, TRIR_f, TRIR_b], 1).astype(NPBF)
    C0_f = np.where(r[:, None] <= r[None, :], 0.0, -30000.0).astype(f32)
    C0_b = np.where(r[:, None] >= r[None, :], 0.0, -30000.0).astype(f32)
    c0m = np.concatenate([np.tile(C0_f, (1, 4)),
                          np.tile(C0_b, (1, 4))], 1).astype(NPBF)
    ident = np.eye(128, dtype=f32)
    identb = np.eye(128).astype(NPBF)
    onescol = np.ones((128, 1)).astype(NPBF)
    onesrow = np.ones((1, 128)).astype(NPBF)
    A_f = -np.exp(A_log_f.astype(f32))
    A_b = -np.exp(A_log_b.astype(f32))

    def pack_win(wc):
        # wc: [1288, 1024] rows in m-tile order -> [128, 8*1288] col-blocks
        blocks = []
        for m in [10] + list(range(10)):
            mr = 8 if m == 10 else 128
            r0 = 1280 if m == 10 else m * 128
            blk = wc[r0:r0 + mr, :].reshape(mr, 8, 128)       # c, k, p
            blocks.append(blk.transpose(2, 1, 0).reshape(128, 8 * mr))
        return np.ascontiguousarray(np.concatenate(blocks, 1)).astype(NPBF)

    def pack_wo(w):
        # w: [2048, 1024] -> [128, m(8) x k(16) x c(128)]
        w4 = w.reshape(16, 128, 8, 128)                # k, p, m, c
        return np.ascontiguousarray(
            w4.transpose(1, 2, 0, 3).reshape(128, 16384)).astype(NPBF)

    def pack_wf(w):
        # w: [1024, 1024] -> [128, m2(8) x k(8) x c(128)]
        w4 = w.reshape(8, 128, 8, 128)                 # k, p, m2, c
        return np.ascontiguousarray(
            w4.transpose(1, 2, 0, 3).reshape(128, 8192)).astype(NPBF)

    in_maps = []
    for i in range(NCORES):
        hs = slice(4 * i, 4 * i + 4)        # heads of this core
        zs = slice(256 * i, 256 * i + 256)  # z/x row slice
        rows = [W_in_f[zs], W_in_b[zs],
                W_in_f[2048 + 256 * i: 2048 + 256 * i + 256],
                W_in_b[2048 + 256 * i: 2048 + 256 * i + 256],
                W_in_f[4096:4224], W_in_b[4096:4224],
                W_in_f[4224 + 4 * i: 4224 + 4 * i + 4],
                W_in_b[4224 + 4 * i: 4224 + 4 * i + 4]]
        # m-tile order: zf(2) zb(2) xf(2) xb(2) BCf BCb dt
        wc = np.concatenate(rows, 0).astype(f32)
        winT = pack_win(wc)
        cw = np.zeros((128, 24), f32)
        cb = np.zeros((128, 6), f32)
        cw_rows = [conv_w_f[zs.start:zs.stop], conv_w_b[zs.start:zs.stop],
                   conv_w_f[2048:2176], conv_w_b[2048:2176]]
        cb_rows = [conv_b_f[zs.start:zs.stop], conv_b_b[zs.start:zs.stop],
                   conv_b_f[2048:2176], conv_b_b[2048:2176]]
        # tiles: xf0 xf1 xb0 xb1 BCf BCb
        tmap = [(0, 0), (0, 1), (1, 0), (1, 1), (2, 0), (3, 0)]
        for t, (src, half) in enumerate(tmap):
            cw[:, t * 4:(t + 1) * 4] = cw_rows[src][half * 128:(half + 1) * 128]
            cb[:, t] = cb_rows[src][half * 128:(half + 1) * 128]
        dtbias = np.concatenate([dt_bias_f[hs], dt_bias_b[hs]]).astype(f32)[:, None]
        arow = np.broadcast_to(
            np.concatenate([A_f[hs], A_b[hs]])[None, :], (128, 8)).astype(f32)
        dcol = np.zeros((128, 4), f32)
        for xt in range(4):
            Dv = D_f if xt < 2 else D_b
            base = 4 * i + (xt % 2) * 2
            dcol[0:64, xt] = Dv[base]
            dcol[64:128, xt] = Dv[base + 1]
        if i < 4:
            woT = pack_wo((W_out_f * norm_w_f[None, :]).T.astype(f32))
            wfT = pack_wf(W_out[:, :1024].T.astype(f32))
        else:
            woT = pack_wo((W_out_b * norm_w_b[None, :]).T.astype(f32))
            wfT = pack_wf(W_out[:, 1024:].T.astype(f32))
        in_maps.append({
            "uT": uT, "winT": winT, "convw": cw, "convb": cb,
            "dtbias": np.ascontiguousarray(dtbias),
            "arow": np.ascontiguousarray(arow),
            "dcol": dcol, "tri": tri, "c0m": c0m, "ident": ident,
            "identb": identb, "onescol": onescol, "onesrow": onesrow,
            "woT": woT, "wfT": wfT,
        })
    return in_maps


def _assemble(results):
    finT = np.zeros((1024, 1024), np.float32)
    for j in range(4):
        op_f = results[j]["out_part"]
        op_b = results[4 + j]["out_part"]
        finT[:, j * 128:(j + 1) * 128] += op_f[:, 0:128]
        finT[:, (j + 4) * 128:(j + 5) * 128] += op_f[:, 128:256]
        finT[:, (j + 4) * 128:(j + 5) * 128] += op_b[:, 0:128]
        finT[:, j * 128:(j + 1) * 128] += op_b[:, 128:256]
    return np.ascontiguousarray(finT.T)[None]


def kernel(**inputs):
    if "nc" not in _CACHE:
        _CACHE["nc"] = _build()
    nc = _CACHE["nc"]
    in_maps = _prep_inputs(**inputs)
    res = run_bass_kernel_spmd(nc, in_maps, core_ids=list(range(NCORES)))
    return _assemble(res.results)


if __name__ == "__main__":
    d = np.load('/root/problem/ref_data.npz')
    inputs = {k: d[k] for k in d.files if k != 'expect'}
    out = kernel(**inputs)
    expect = d['expect']
    err = np.abs(out - expect).max()
    print(f"absmax err {err:.3e}  rel {err / np.abs(expect).max():.3e}")


# revision 18
# speedup vs baseline: 1.0307x; 1.0307x over previous
"""Trainium2 Bass kernel for bidirectional Mamba2 (DSSMamba2), 8 NeuronCores.

Strategy (single SPMD launch, all-bf16 matmul datapath):
  Phase 1 (all 8 cores): each core owns 4 fwd + 4 bwd heads. Computes its
  in_proj slice (channels-on-partitions, bf16 weights loaded once per
  m-tile in a host-prearranged contiguous layout), depthwise causal /
  anti-causal conv + silu, softplus(dt), and the chunked-SSD scan
  (8 chunks of 128) for its heads. Backward direction is evaluated in
  reverse ("true") time so no flips are ever materialized. ln(dt_s) is
  folded into the exp() bias so T = G * exp(cum_t - cum_s + ln dt_s + C0)
  needs one ACT exp + one DVE mult per head-chunk. Gated outputs
  yg = y * silu(z) are written per chunk as bf16 and redistributed
  head-split -> L-split with two bf16 AllToAlls.
  Phase 2 runs per piece (gated RMSNorm, out_proj, silu, final
  projection); piece 0 starts as soon as its AllToAll lands, hiding the
  second AllToAll. out_proj / final weights are preloaded into SBUF at
  kernel start so phase 2 is pure compute.
  Host: reorders/adds the per-core partials (pure gather/unshard).
"""
import sys
if '/opt/trn_rl_repo' not in sys.path:
    sys.path.insert(0, '/opt/trn_rl_repo')

import numpy as np
import ml_dtypes
import concourse.bass as bass
import concourse.tile as tile
from concourse import bacc, mybir
from concourse.bass_utils import run_bass_kernel_spmd

F32 = mybir.dt.float32
BF16 = mybir.dt.bfloat16
AOP = mybir.AluOpType
ACT = mybir.ActivationFunctionType
NPBF = np.dtype(ml_dtypes.bfloat16)

L = 1024
DM = 1024            # d_model
Q = 128              # chunk length
NCH = 8              # number of chunks
P = 64               # headdim
NCORES = 8
CIN = 1288           # per-core in_proj rows: 2*(256 z + 256 x + 128 BC + 4 dt)
EPS = 1e-5
D_INNER = 2048

_CACHE = {}


def _build():
    nc = bacc.Bacc("TRN2", target_bir_lowering=False, debug=False)

    # ---- DRAM I/O ----
    uT = nc.dram_tensor("uT", [DM, L], BF16, kind="ExternalInput")
    # winT cols: m-blocks in order [dt(8), m0..m9(128 each)], each block
    # k-major: col = off(m) + k*mrows + c
    winT = nc.dram_tensor("winT", [128, 8 * CIN], BF16, kind="ExternalInput")
    convw = nc.dram_tensor("convw", [128, 24], F32, kind="ExternalInput")
    convb = nc.dram_tensor("convb", [128, 6], F32, kind="ExternalInput")
    dtbias = nc.dram_tensor("dtbias", [8, 1], F32, kind="ExternalInput")
    arow = nc.dram_tensor("arow", [128, 8], F32, kind="ExternalInput")
    dcol = nc.dram_tensor("dcol", [128, 4], F32, kind="ExternalInput")
    tri = nc.dram_tensor("tri", [128, 512], BF16, kind="ExternalInput")
    # tri columns: [TRI_f | TRI_b | TRIR_f | TRIR_b]
    c0m = nc.dram_tensor("c0m", [128, 1024], BF16, kind="ExternalInput")
    # c0m: [C0_f x4 | C0_b x4]  (per-head-tiled mask)
    ident = nc.dram_tensor("ident", [128, 128], F32, kind="ExternalInput")
    identb = nc.dram_tensor("identb", [128, 128], BF16, kind="ExternalInput")
    onescol = nc.dram_tensor("onescol", [128, 1], BF16, kind="ExternalInput")
    onesrow = nc.dram_tensor("onesrow", [1, 128], BF16, kind="ExternalInput")
    # woT cols: m(8) x k(16) x c(128); wfT cols: m2(8) x k(8) x c(128)
    woT = nc.dram_tensor("woT", [128, 16384], BF16, kind="ExternalInput")
    wfT = nc.dram_tensor("wfT", [128, 8192], BF16, kind="ExternalInput")
    out_part = nc.dram_tensor("out_part", [DM, 256], F32, kind="ExternalOutput")

    # collective bounce buffers (internal DRAM)
    a2a_in = [nc.dram_tensor(f"a2a_in{p}", [NCORES, 256, Q], BF16)
              for p in range(2)]
    a2a_out = [nc.dram_tensor(f"a2a_out{p}", [NCORES, 256, Q], BF16)
               for p in range(2)]

    with tile.TileContext(nc) as tc, \
            nc.allow_low_precision(reason="bf16 datapath within 2e-2 budget"):
        import contextlib
        ctx = contextlib.ExitStack()
        sb = ctx.enter_context(tc.tile_pool(name="sb", bufs=1))
        scr = ctx.enter_context(tc.tile_pool(name="scr", bufs=4))
        scs = ctx.enter_context(tc.tile_pool(name="scs", bufs=6))
        zpool = ctx.enter_context(tc.tile_pool(name="zpool", bufs=4))
        upool = ctx.enter_context(tc.tile_pool(name="upool", bufs=4))
        blpool = ctx.enter_context(tc.tile_pool(name="blpool", bufs=2))
        bcp = ctx.enter_context(tc.tile_pool(name="bcp", bufs=4))
        w_pool = ctx.enter_context(tc.tile_pool(name="wstream", bufs=2))
        uT_pool = ctx.enter_context(tc.tile_pool(name="uTp", bufs=8))
        yga_pool = ctx.enter_context(tc.tile_pool(name="yga", bufs=24))
        ps_mm = ctx.enter_context(tc.tile_pool(name="ps_mm", bufs=2, space="PSUM"))
        ps_gt = ctx.enter_context(tc.tile_pool(name="ps_gt", bufs=2, space="PSUM"))
        ps_y = ctx.enter_context(tc.tile_pool(name="ps_y", bufs=2, space="PSUM"))
        ps_cc = ctx.enter_context(tc.tile_pool(name="ps_cc", bufs=1, space="PSUM"))

        # ---- uT into SBUF (scalar ring, ahead of the weight preload) ----
        uT_t = []
        for k in range(8):
            t = uT_pool.tile([128, 1024], BF16, tag="uT", name=f"uTk{k}")
            nc.scalar.dma_start(out=t[:], in_=uT[k * 128:(k + 1) * 128, :])
            uT_t.append(t)

        # ---- phase-2 weights preloaded (stream during phase 1) ----
        wo_sb = sb.tile([128, 16384], BF16, tag="wo")
        wf_sb = sb.tile([128, 8192], BF16, tag="wf")
        for q in range(4):
            nc.scalar.dma_start(out=wo_sb[:, q * 4096:(q + 1) * 4096],
                                in_=woT[:, q * 4096:(q + 1) * 4096])
            nc.scalar.dma_start(out=wf_sb[:, q * 2048:(q + 1) * 2048],
                                in_=wfT[:, q * 2048:(q + 1) * 2048])


        # ---- constants into SBUF ----
        tri_t = sb.tile([128, 512], BF16, tag="tri")
        nc.sync.dma_start(out=tri_t[:], in_=tri[:])
        c0_t = sb.tile([128, 1024], BF16, tag="c0")
        nc.sync.dma_start(out=c0_t[:], in_=c0m[:])
        id_t = sb.tile([128, 128], F32, tag="id")
        nc.sync.dma_start(out=id_t[:], in_=ident[:])
        idb_t = sb.tile([128, 128], BF16, tag="idb")
        nc.sync.dma_start(out=idb_t[:], in_=identb[:])
        ones_t = sb.tile([128, 1], BF16, tag="ones")
        nc.sync.dma_start(out=ones_t[:], in_=onescol[:])
        onesr_t = sb.tile([1, 128], BF16, tag="onesr")
        nc.sync.dma_start(out=onesr_t[:], in_=onesrow[:])
        cw_t = sb.tile([128, 24], F32, tag="cw")
        nc.sync.dma_start(out=cw_t[:], in_=convw[:])
        cb_t = sb.tile([128, 6], F32, tag="cb")
        nc.sync.dma_start(out=cb_t[:], in_=convb[:])
        dtb_t = sb.tile([8, 1], F32, tag="dtb")
        nc.sync.dma_start(out=dtb_t[:], in_=dtbias[:])
        ar_t = sb.tile([128, 8], F32, tag="ar")
        nc.sync.dma_start(out=ar_t[:], in_=arow[:])
        dc_t = sb.tile([128, 4], F32, tag="dc")
        nc.sync.dma_start(out=dc_t[:], in_=dcol[:])
        one8 = sb.tile([8, 1], F32, tag="one8")
        nc.vector.memset(one8[:], 1.0)

        TRI_f = tri_t[:, 0:128]
        TRI_b = tri_t[:, 128:256]
        TRIR_f = tri_t[:, 256:384]
        TRIR_b = tri_t[:, 384:512]
        C04 = [c0_t[:, 0:512], c0_t[:, 512:1024]]

        # ---- persistent phase-1 tiles ----
        z_sb = [zpool.tile([128, L], BF16, tag="z", name=f"z{t}")
                for t in range(4)]
        xpad = [sb.tile([128, L + 3], BF16, tag=f"xpad{t}", name=f"xpad{t}")
                for t in range(6)]
        xact = [sb.tile([128, L], BF16, tag=f"xact{t}", name=f"xact{t}")
                for t in range(6)]
        cact = [sb.tile([64, L], BF16, tag=f"cact{d}", name=f"cact{d}")
                for d in range(2)]
        ygs = [sb.tile([128, L], BF16, tag=f"ygs{t}", name=f"ygs{t}")
               for t in range(4)]
        dt_sb = sb.tile([8, L], F32, tag="dt")
        cum8 = sb.tile([8, L], F32, tag="cum8")
        wrev8 = sb.tile([8, L], F32, tag="wrev8")
        lndt8 = sb.tile([8, L], F32, tag="lndt8")
        sd8 = sb.tile([8, L], F32, tag="sd8")
        omdt8 = sb.tile([8, L], BF16, tag="omdt8")
        ecum8 = sb.tile([8, L], BF16, tag="ecum8")
        ecum8f = sb.tile([8, L], F32, tag="ecum8f")
        edrow = sb.tile([1, 64], F32, tag="edrow")
        edall = sb.tile([64, 64], F32, tag="edall")
        sdT_sb = [sb.tile([128, 8], F32, tag=f"sdT{c}", name=f"sdT{c}")
                  for c in range(NCH)]
        omdtT_sb = [sb.tile([128, 8], F32, tag=f"om{c}", name=f"om{c}")
                    for c in range(NCH)]
        S_sb = [sb.tile([64, 64], BF16, tag=f"S{h}", name=f"S{h}")
                for h in range(8)]

        # zero-pad columns of conv inputs
        for t in range(6):
            fwd = t in (0, 1, 4)
            if fwd:
                nc.vector.memset(xpad[t][:, 0:3], 0.0)
            else:
                nc.vector.memset(xpad[t][:, L:L + 3], 0.0)
        for h in range(8):
            nc.vector.memset(S_sb[h][:], 0.0)

        # ---- in_proj: out^T[m-tile, L] = winT.T @ uT ----
        # m-tile map: 0-1 zf, 2-3 zb, 4-5 xf, 6-7 xb, 8 BCf, 9 BCb, 10 dt(8)
        off = {}
        o = 0
        for m in [10] + list(range(10)):
            off[m] = o
            o += 8 * (8 if m == 10 else 128)
        for m in [10] + list(range(10)):
            mrows = 8 if m == 10 else 128
            wt = w_pool.tile([128, 8, mrows], BF16, tag="w", name="wct")
            nc.sync.dma_start(out=wt[:],
                              in_=winT[:, off[m]:off[m] + 8 * mrows])
            ps = [ps_mm.tile([128, 512], F32, tag="mm", name=f"mmps{nh}")
                  for nh in range(2)]
            for k in range(8):
                for nh in range(2):
                    nc.tensor.matmul(
                        ps[nh][0:mrows, :], wt[:, k, :],
                        uT_t[k][:, nh * 512:(nh + 1) * 512],
                        start=(k == 0), stop=(k == 7))
            for nh in range(2):
                nsl = slice(nh * 512, (nh + 1) * 512)
                if m == 10:     # dt: softplus(dt + bias) = ln(1 + exp(.))
                    spe = scr.tile([8, 512], F32, tag="scr8", name="spe", bufs=2)
                    nc.scalar.activation(
                        out=spe[:], in_=ps[nh][0:8, :],
                        func=ACT.Exp, bias=dtb_t[:], scale=1.0)
                    nc.scalar.activation(
                        out=dt_sb[:, nsl], in_=spe[:],
                        func=ACT.Ln, bias=one8[:], scale=1.0)
                elif m < 4:     # z
                    nc.scalar.activation(out=z_sb[m][:, nsl], in_=ps[nh][:],
                                         func=ACT.Copy)
                else:           # xBC -> padded tile
                    t = m - 4
                    po = 3 if t in (0, 1, 4) else 0
                    nc.scalar.activation(
                        out=xpad[t][:, po + nh * 512: po + (nh + 1) * 512],
                        in_=ps[nh][:], func=ACT.Copy)

        # ---- silu(z) in place (z is only ever used gated) ----
        for t in range(4):
            for hp2 in range(2):
                zsl = slice(hp2 * 512, (hp2 + 1) * 512)
                sgz = scr.tile([128, 512], BF16, tag="scr", name="sgz")
                nc.scalar.activation(out=sgz[:], in_=z_sb[t][:, zsl],
                                     func=ACT.Sigmoid)
                nc.vector.tensor_tensor(out=z_sb[t][:, zsl],
                                        in0=z_sb[t][:, zsl], in1=sgz[:],
                                        op=AOP.mult)

        # ---- conv + silu ----
        for t in range(6):
            fwd = t in (0, 1, 4)
            eng = nc.vector
            cvo = scr.tile([128, L], BF16, tag="scr", name="cvo")
            first = True
            for w in range(4):
                po = w if fwd else 3 - w
                src = xpad[t][:, po:po + L]
                wcol = cw_t[:, t * 4 + w:t * 4 + w + 1]
                if first:
                    eng.tensor_scalar(
                        out=cvo[:], in0=src, scalar1=wcol,
                        scalar2=cb_t[:, t:t + 1], op0=AOP.mult, op1=AOP.add)
                    first = False
                else:
                    eng.scalar_tensor_tensor(
                        out=cvo[:], in0=src, scalar=wcol, in1=cvo[:],
                        op0=AOP.mult, op1=AOP.add)
            nc.scalar.activation(out=xact[t][:], in_=cvo[:], func=ACT.Sigmoid)
            nc.vector.tensor_tensor(out=xact[t][:], in0=xact[t][:],
                                    in1=cvo[:], op=AOP.mult)
            if t >= 4:
                nc.sync.dma_start(out=cact[t - 4][:], in_=xact[t][64:128, :])

        # ---- dt preprocessing ----
        nc.scalar.activation(out=lndt8[:], in_=dt_sb[:], func=ACT.Ln)
        for c in range(NCH):
            csl = slice(c * Q, (c + 1) * Q)
            tp = ps_gt.tile([128, 128], F32, tag="gt", name="tp")
            nc.tensor.transpose(tp[:, 0:8], dt_sb[:, csl], id_t[0:8, 0:8])
            ldT = scs.tile([128, 8], BF16, tag="ldT", name="ldT")
            nc.vector.tensor_tensor(out=ldT[:], in0=tp[:, 0:8],
                                    in1=ar_t[:], op=AOP.mult)
            ccps = ps_cc.tile([4, 512], F32, tag="cc", name="ccps")
            nc.tensor.matmul(ccps[0:4, 0:128], ldT[:, 0:4], TRI_f,
                             start=True, stop=True)
            nc.tensor.matmul(ccps[0:4, 128:256], ldT[:, 0:4], TRIR_f,
                             start=True, stop=True)
            nc.tensor.matmul(ccps[0:4, 256:384], ldT[:, 4:8], TRI_b,
                             start=True, stop=True)
            nc.tensor.matmul(ccps[0:4, 384:512], ldT[:, 4:8], TRIR_b,
                             start=True, stop=True)
            stg = scr.tile([4, 512], F32, tag="stg", name="stg", bufs=2)
            nc.vector.tensor_copy(out=stg[:], in_=ccps[0:4, :])
            # rows 4:8 are partition-unaligned for engines; DMA is exempt
            nc.sync.dma_start(out=cum8[0:4, csl], in_=stg[0:4, 0:128])
            nc.sync.dma_start(out=cum8[4:8, csl], in_=stg[0:4, 256:384])
            nc.sync.dma_start(out=wrev8[0:4, csl], in_=stg[0:4, 128:256])
            nc.sync.dma_start(out=wrev8[4:8, csl], in_=stg[0:4, 384:512])
        # sd = ln(dt) - cum ; omdt = exp(wrev + ln dt) ; ecum = exp(cum)
        nc.vector.tensor_tensor(out=sd8[:], in0=lndt8[:], in1=cum8[:],
                                op=AOP.subtract)
        nc.vector.tensor_tensor(out=wrev8[:], in0=wrev8[:], in1=lndt8[:],
                                op=AOP.add)
        nc.scalar.activation(out=omdt8[:], in_=wrev8[:], func=ACT.Exp)
        nc.scalar.activation(out=ecum8[:], in_=cum8[:], func=ACT.Exp)
        nc.scalar.activation(out=ecum8f[:], in_=cum8[:], func=ACT.Exp)
        # per-chunk decay scalars exp(cum at boundary); fwd boundary is
        # chunk position 127, bwd position 0. Gather all 64 (c,h) values
        # into one row, broadcast once.
        for c in range(NCH):
            nc.sync.dma_start(
                out=edrow[0:1, c * 8:c * 8 + 4],
                in_=ecum8f[0:4, c * Q + Q - 1:c * Q + Q])
            nc.sync.dma_start(
                out=edrow[0:1, c * 8 + 4:c * 8 + 8],
                in_=ecum8f[4:8, c * Q:c * Q + 1])
        nc.gpsimd.partition_broadcast(edall[:], edrow[0:1, :])
        for c in range(NCH):
            csl = slice(c * Q, (c + 1) * Q)
            tps = ps_gt.tile([128, 128], F32, tag="gt", name="tps")
            nc.tensor.transpose(tps[:, 0:8], sd8[:, csl], id_t[0:8, 0:8])
            nc.vector.tensor_copy(out=sdT_sb[c][:], in_=tps[:, 0:8])
            tpo = ps_gt.tile([128, 128], BF16, tag="gt", name="tpo")
            nc.tensor.transpose(tpo[:, 0:8], omdt8[:, csl], idb_t[0:8, 0:8])
            nc.vector.tensor_copy(out=omdtT_sb[c][:], in_=tpo[:, 0:8])

        # ---- chunked SSD scan ----
        def make_bcast(d, c):
            """Per-head cum + exp(cum) rows for chunk c broadcast to 128
            partitions. bc4 additionally gets the C0 causal mask added."""
            csl = slice(c * Q, (c + 1) * Q)
            bc = bcp.tile([128, 512], F32, tag="bc", name="bc")
            eb = bcp.tile([128, 512], BF16, tag="eb", name="eb")
            nc.sync.dma_start(out=bc[0:1, 0:512],
                              in_=cum8[d * 4:d * 4 + 4, csl])
            nc.sync.dma_start(out=eb[0:1, 0:512],
                              in_=ecum8[d * 4:d * 4 + 4, csl])
            nc.gpsimd.partition_broadcast(bc[:], bc[0:1, :])
            nc.gpsimd.partition_broadcast(eb[:], eb[0:1, :])
            # add the causal mask (-30000 outside) in place
            nc.vector.tensor_tensor(out=bc[:], in0=bc[:], in1=C04[d],
                                    op=AOP.add)
            return bc, eb

        def scan_chunk(d, c, first, bc, eb):
            """Process direction d (0 fwd, 1 bwd) chunk c (true time)."""
            csl = slice(c * Q, (c + 1) * Q)
            BC = xact[4 + d]
            # B transpose (L-on-partitions) for state accumulation
            btp = ps_gt.tile([128, 128], BF16, tag="gt", name="btp")
            nc.tensor.transpose(btp[:, 0:64], BC[0:64, csl],
                                idb_t[0:64, 0:64])
            bl = blpool.tile([128, 64], BF16, tag="bl", name="bl")
            nc.scalar.activation(out=bl[:], in_=btp[:, 0:64], func=ACT.Copy)
            # x^T transposes
            u_t = []
            for xi in range(2):
                xt = 2 * d + xi
                up = ps_gt.tile([128, 128], BF16, tag="gt", name="up")
                nc.tensor.transpose(up[:], xact[xt][:, csl], idb_t[:])
                ut = upool.tile([128, 128], BF16, tag="u", name="ut")
                nc.scalar.activation(out=ut[:], in_=up[:], func=ACT.Copy)
                u_t.append(ut)
            # G (shared across the dir's heads)
            gps = ps_gt.tile([128, 128], F32, tag="gt", name="gps")
            nc.tensor.matmul(gps[:], BC[0:64, csl], cact[d][:, csl],
                             start=True, stop=True)
            gsb = scs.tile([128, 128], BF16, tag="gsb", name="gsb")
            nc.scalar.activation(out=gsb[:], in_=gps[:], func=ACT.Copy)
            # batched states for all 4 heads: uwall = x^T * (dt*omega)
            uwall = scr.tile([128, 256], BF16, tag="scrh", name="uwall")
            for hl in range(4):
                col = d * 4 + hl
                nc.vector.tensor_scalar_mul(
                    out=uwall[:, hl * 64:(hl + 1) * 64],
                    in0=u_t[hl // 2][:, (hl % 2) * 64:(hl % 2) * 64 + 64],
                    scalar1=omdtT_sb[c][:, col:col + 1])
            sps4 = ps_y.tile([64, 256], F32, tag="st", name="sps4", bufs=1)
            nc.tensor.matmul(sps4[:], bl[:], uwall[:], start=True, stop=True)
            for xi in range(2):
                xt = 2 * d + xi
                yful = ps_y.tile([128, 128], F32, tag="y", name="yful")
                for hh in range(2):
                    col = d * 4 + xi * 2 + hh
                    hl4 = xi * 2 + hh
                    hsl = slice(hh * 64, (hh + 1) * 64)
                    # T = G * exp(cum_t - cum_s + ln dt_s + C0)
                    ee = scs.tile([128, 128], BF16, tag="ee", name="ee")
                    nc.scalar.activation(
                        out=ee[:], in_=bc[:, hl4 * Q:(hl4 + 1) * Q],
                        func=ACT.Exp, bias=sdT_sb[c][:, col:col + 1],
                        scale=1.0)
                    tt = scs.tile([128, 128], BF16, tag="tt", name="ttl")
                    nc.vector.tensor_tensor(out=tt[:], in0=ee[:],
                                            in1=gsb[:], op=AOP.mult)
                    # Y^T = U^T T (+ S^T (C*ecum))
                    nc.tensor.matmul(yful[hsl, :], u_t[xi][:, hsl], tt[:],
                                     start=True, stop=first)
                    if not first:
                        cem = scs.tile([64, 128], BF16, tag="cem",
                                       name="cem")
                        nc.vector.tensor_tensor(
                            out=cem[:], in0=cact[d][:, csl],
                            in1=eb[0:64, hl4 * Q:(hl4 + 1) * Q],
                            op=AOP.mult)
                        nc.tensor.matmul(yful[hsl, :], S_sb[col][:], cem[:],
                                         start=False, stop=True)
                    # S = edecay * S + states  (after Y read S)
                    nc.vector.scalar_tensor_tensor(
                        out=S_sb[col][:], in0=S_sb[col][:],
                        scalar=edall[:, c * 8 + col:c * 8 + col + 1],
                        in1=sps4[:, hl4 * 64:(hl4 + 1) * 64],
                        op0=AOP.mult, op1=AOP.add)
                # y = Y + D*x, then gate with silu(z), bf16 out
                ysc = scs.tile([128, 128], BF16, tag="ysc", name="ysc")
                nc.vector.scalar_tensor_tensor(
                    out=ysc[:], in0=xact[xt][:, csl],
                    scalar=dc_t[:, xt:xt + 1], in1=yful[:],
                    op0=AOP.mult, op1=AOP.add)
                nc.vector.tensor_tensor(out=ygs[xt][:, csl], in0=ysc[:],
                                        in1=z_sb[xt][:, csl], op=AOP.mult)

        def emit_piece(p):
            """Stage already-gated yg columns, launch AllToAll."""
            # piece 0: fwd chunks 0-3 to cores 0-3, bwd chunks 4-7 to 4-7
            for k in range(NCORES):
                if k < 4:
                    ch = k if p == 0 else k + 4     # fwd chunk
                    t0, t1 = 0, 1
                else:
                    j = k - 4
                    ch = (j + 4) if p == 0 else j   # bwd chunk
                    t0, t1 = 2, 3
                nc.scalar.dma_start(out=a2a_in[p][k, 0:128, :],
                                    in_=ygs[t0][:, ch * Q:(ch + 1) * Q])
                nc.scalar.dma_start(out=a2a_in[p][k, 128:256, :],
                                    in_=ygs[t1][:, ch * Q:(ch + 1) * Q])
            nc.gpsimd.collective_compute(
                "AllToAll", AOP.bypass,
                replica_groups=[list(range(NCORES))],
                ins=[a2a_in[p].ap().opt()], outs=[a2a_out[p].ap().opt()])

        bc_next = [make_bcast(0, 0), make_bcast(1, NCH - 1)]
        for step in range(NCH):
            bc_cur = bc_next
            if step + 1 < NCH:
                bc_next = [make_bcast(0, step + 1),
                           make_bcast(1, NCH - 2 - step)]
            scan_chunk(0, step, first=(step == 0),
                       bc=bc_cur[0][0], eb=bc_cur[0][1])
            scan_chunk(1, NCH - 1 - step, first=(step == 0),
                       bc=bc_cur[1][0], eb=bc_cur[1][1])
            if step == 3:
                emit_piece(0)
        emit_piece(1)

        # ---- phase 2 (per piece): receive, norm, out_proj, silu, final ----
        epst = sb.tile([1, 1], F32, tag="epst")
        nc.vector.memset(epst[:], EPS)
        for p in range(2):
            ygall = []
            for g in range(16):
                t = yga_pool.tile([128, 128], BF16, tag="yga",
                                  name=f"yga{p}_{g}")
                nc.scalar.dma_start(
                    out=t[:], in_=a2a_out[p][g // 2,
                                             (g % 2) * 128:(g % 2) * 128 + 128,
                                             :])
                ygall.append(t)
            ssps = ps_cc.tile([1, 512], F32, tag="cc", name="ssps")
            for g in range(16):
                sq = scs.tile([128, 128], BF16, tag="sq", name="sq")
                nc.vector.tensor_tensor(out=sq[:], in0=ygall[g][:],
                                        in1=ygall[g][:], op=AOP.mult)
                nc.tensor.matmul(ssps[0:1, 0:128], ones_t[:], sq[:],
                                 start=(g == 0), stop=(g == 15))
            rs = scs.tile([1, 128], BF16, tag="rs", name="rs")
            rsf = scs.tile([1, 128], F32, tag="rsf", name="rsf")
            nc.scalar.activation(out=rsf[:], in_=ssps[0:1, 0:128],
                                 func=ACT.Sqrt, bias=epst[:],
                                 scale=1.0 / D_INNER)
            nc.vector.reciprocal(out=rs[:], in_=rsf[:])
            rsb = scs.tile([128, 128], BF16, tag="rsb", name="rsb")
            nc.gpsimd.partition_broadcast(rsb[:], rs[0:1, :])

            silu_sb = []
            for m in range(8):
                ps = ps_mm.tile([128, 128], F32, tag="mm", name="mm2")
                for kk in range(16):
                    nc.tensor.matmul(
                        ps[:], wo_sb[:, m * 2048 + kk * 128:
                                     m * 2048 + (kk + 1) * 128],
                        ygall[kk][:], start=(kk == 0), stop=(kk == 15))
                t1 = scs.tile([128, 128], BF16, tag="t1", name="t1")
                nc.vector.tensor_tensor(out=t1[:], in0=ps[:], in1=rsb[:],
                                        op=AOP.mult)
                sg = scs.tile([128, 128], BF16, tag="sg", name="sg")
                nc.scalar.activation(out=sg[:], in_=t1[:], func=ACT.Sigmoid)
                st = yga_pool.tile([128, 128], BF16, tag="yga",
                                   name=f"st{p}_{m}")
                nc.vector.tensor_tensor(out=st[:], in0=sg[:],
                                        in1=t1[:], op=AOP.mult)
                silu_sb.append(st)

            for m2 in range(8):
                ps = ps_mm.tile([128, 128], F32, tag="mm", name="mm3")
                for m in range(8):
                    nc.tensor.matmul(
                        ps[:], wf_sb[:, m2 * 1024 + m * 128:
                                     m2 * 1024 + (m + 1) * 128],
                        silu_sb[m][:], start=(m == 0), stop=(m == 7))
                ot = scs.tile([128, 128], F32, tag="ot", name="ot")
                nc.scalar.activation(out=ot[:], in_=ps[:], func=ACT.Copy)
                nc.sync.dma_start(
                    out=out_part[m2 * 128:(m2 + 1) * 128,
                                 p * 128:(p + 1) * 128],
                    in_=ot[:])
        ctx.close()

    nc.compile()
    return nc


def _prep_inputs(u, W_in_f, W_in_b, conv_w_f, conv_b_f, conv_w_b, conv_b_b,
                 dt_bias_f, dt_bias_b, A_log_f, A_log_b, D_f, D_b,
                 norm_w_f, norm_w_b, W_out_f, W_out_b, W_out):
    f32 = np.float32
    uT = np.ascontiguousarray(u[0].T).astype(NPBF)
    r = np.arange(Q)
    TRI_f = (r[:, None] <= r[None, :]).astype(f32)
    TRI_b = (r[:, None] >= r[None, :]).astype(f32)
    TRIR_f = (r[:, None] > r[None, :]).astype(f32)
    TRIR_b = (r[:, None] < r[None, :]).astype(f32)
    tri = np.concatenate([TRI_f, TRI_b, TRIR_f, TRIR_b], 1).astype(NPBF)
    C0_f = np.where(r[:, None] <= r[None, :], 0.0, -30000.0).astype(f32)
    C0_b = np.where(r[:, None] >= r[None, :], 0.0, -30000.0).astype(f32)
    c0m = np.concatenate([np.tile(C0_f, (1, 4)),
                          np.tile(C0_b, (1, 4))], 1).astype(NPBF)
    ident = np.eye(128, dtype=f32)
    identb = np.eye(128).astype(NPBF)
    onescol = np.ones((128, 1)).astype(NPBF)
    onesrow = np.ones((1, 128)).astype(NPBF)
    A_f = -np.exp(A_log_f.astype(f32))
    A_b = -np.exp(A_log_b.astype(f32))

    def pack_win(wc):
        # wc: [1288, 1024] rows in m-tile order -> [128, 8*1288] col-blocks
        blocks = []
        for m in [10] + list(range(10)):
            mr = 8 if m == 10 else 128
            r0 = 1280 if m == 10 else m * 128
            blk = wc[r0:r0 + mr, :].reshape(mr, 8, 128)       # c, k, p
            blocks.append(blk.transpose(2, 1, 0).reshape(128, 8 * mr))
        return np.ascontiguousarray(np.concatenate(blocks, 1)).astype(NPBF)

    def pack_wo(w):
        # w: [2048, 1024] -> [128, m(8) x k(16) x c(128)]
        w4 = w.reshape(16, 128, 8, 128)                # k, p, m, c
        return np.ascontiguousarray(
            w4.transpose(1, 2, 0, 3).reshape(128, 16384)).astype(NPBF)

    def pack_wf(w):
        # w: [1024, 1024] -> [128, m2(8) x k(8) x c(128)]
        w4 = w.reshape(8, 128, 8, 128)                 # k, p, m2, c
        return np.ascontiguousarray(
            w4.transpose(1, 2, 0, 3).reshape(128, 8192)).astype(NPBF)

    in_maps = []
    for i in range(NCORES):
        hs = slice(4 * i, 4 * i + 4)        # heads of this core
        zs = slice(256 * i, 256 * i + 256)  # z/x row slice
        rows = [W_in_f[zs], W_in_b[zs],
                W_in_f[2048 + 256 * i: 2048 + 256 * i + 256],
                W_in_b[2048 + 256 * i: 2048 + 256 * i + 256],
                W_in_f[4096:4224], W_in_b[4096:4224],
                W_in_f[4224 + 4 * i: 4224 + 4 * i + 4],
                W_in_b[4224 + 4 * i: 4224 + 4 * i + 4]]
        # m-tile order: zf(2) zb(2) xf(2) xb(2) BCf BCb dt
        wc = np.concatenate(rows, 0).astype(f32)
        winT = pack_win(wc)
        cw = np.zeros((128, 24), f32)
        cb = np.zeros((128, 6), f32)
        cw_rows = [conv_w_f[zs.start:zs.stop], conv_w_b[zs.start:zs.stop],
                   conv_w_f[2048:2176], conv_w_b[2048:2176]]
        cb_rows = [conv_b_f[zs.start:zs.stop], conv_b_b[zs.start:zs.stop],
                   conv_b_f[2048:2176], conv_b_b[2048:2176]]
        # tiles: xf0 xf1 xb0 xb1 BCf BCb
        tmap = [(0, 0), (0, 1), (1, 0), (1, 1), (2, 0), (3, 0)]
        for t, (src, half) in enumerate(tmap):
            cw[:, t * 4:(t + 1) * 4] = cw_rows[src][half * 128:(half + 1) * 128]
            cb[:, t] = cb_rows[src][half * 128:(half + 1) * 128]
        dtbias = np.concatenate([dt_bias_f[hs], dt_bias_b[hs]]).astype(f32)[:, None]
        arow = np.broadcast_to(
            np.concatenate([A_f[hs], A_b[hs]])[None, :], (128, 8)).astype(f32)
        dcol = np.zeros((128, 4), f32)
        for xt in range(4):
            Dv = D_f if xt < 2 else D_b
            base = 4 * i + (xt % 2) * 2
            dcol[0:64, xt] = Dv[base]
            dcol[64:128, xt] = Dv[base + 1]
        if i < 4:
            woT = pack_wo((W_out_f * norm_w_f[None, :]).T.astype(f32))
            wfT = pack_wf(W_out[:, :1024].T.astype(f32))
        else:
            woT = pack_wo((W_out_b * norm_w_b[None, :]).T.astype(f32))
            wfT = pack_wf(W_out[:, 1024:].T.astype(f32))
        in_maps.append({
            "uT": uT, "winT": winT, "convw": cw, "convb": cb,
            "dtbias": np.ascontiguousarray(dtbias),
            "arow": np.ascontiguousarray(arow),
            "dcol": dcol, "tri": tri, "c0m": c0m, "ident": ident,
            "identb": identb, "onescol": onescol, "onesrow": onesrow,
            "woT": woT, "wfT": wfT,
        })
    return in_maps


def _assemble(results):
    finT = np.zeros((1024, 1024), np.float32)
    for j in range(4):
        op_f = results[j]["out_part"]
        op_b = results[4 + j]["out_part"]
        finT[:, j * 128:(j + 1) * 128] += op_f[:, 0:128]
        finT[:, (j + 4) * 128:(j + 5) * 128] += op_f[:, 128:256]
        finT[:, (j + 4) * 128:(j + 5) * 128] += op_b[:, 0:128]
        finT[:, j * 128:(j + 1) * 128] += op_b[:, 128:256]
    return np.ascontiguousarray(finT.T)[None]


def kernel(**inputs):
    if "nc" not in _CACHE:
        _CACHE["nc"] = _build()
    nc = _CACHE["nc"]
    in_maps = _prep_inputs(**inputs)
    res = run_bass_kernel_spmd(nc, in_maps, core_ids=list(range(NCORES)))
    return _assemble(res.results)


if __name__ == "__main__":
    d = np.load('/root/problem/ref_data.npz')
    inputs = {k: d[k] for k in d.files if k != 'expect'}
    out = kernel(**inputs)
    expect = d['expect']
    err = np.abs(out - expect).max()
    print(f"absmax err {err:.3e}  rel {err / np.abs(expect).max():.3e}")
